# revision 17
# baseline (speedup 1.0000x reference)
"""Trainium2 Bass kernel for nn_Actor (2x GCNConv + mean-pool + MLP head).

Strategy (8 NeuronCores, SPMD):
  - Nodes are partitioned across the 8 cores (6250 each, padded to 6272).
    Within each core, local nodes are reordered by descending (in-)degree so
    that fixed-slot edge aggregation has minimal padding.
  - Node tensors are stored row-major [node, feature] (node on SBUF
    partitions for on-chip tiles).
  - Per layer: u = (h @ W) * dinv is computed locally, AllGathered so every
    core holds the full u table [50176, 128] f32 in DRAM.
  - Edge aggregation: destinations are processed in windows of 128 (one
    SBUF partition each). Edges into a window are organized into K_w
    "slots"; slot k is one indirect-DMA gather of 128 source rows (dest p
    <- u_full[src of its k-th edge], dummy = zero row). The K_w gathered
    tiles are summed on the TensorEngine by identity-matmul accumulation
    into PSUM - the only per-edge compute.
  - Epilogue fuses relu(s * dinv + b) (ACT, per-partition scale) -> next
    layer's matmul -> u2, and for the final layer the per-graph indicator
    matmul pooling (batch ids -> is_equal one-hots).
  - Pooled sums are AllReduced, the tiny MLP head runs replicated on every
    core; output [64, 8] is identical on all cores.

All index/layout preparation happens on the host (graph structure only);
all floating-point math runs on device.
"""
import sys

sys.path.insert(0, "/opt/trn_rl_repo")

import numpy as np

import concourse.bass as bass
import concourse.mybir as mybir
import concourse.tile as tile
from concourse import bacc
from concourse.bass_utils import run_bass_kernel_spmd

dt = mybir.dt

# Problem constants (hardcoded per spec)
N = 50000
E = 500000
H = 128
G = 64
SV = 64
A = 8
NCORES = 8
NLOC = N // NCORES          # 6250
P = 128
W = 49                      # windows of 128 dests per core
NP = W * P                  # 6272 padded local nodes
NT = NCORES * NP            # 50176 padded global rows in u_full
DUMMY_ROW = 6250            # core-0 pad row (always zero)


# ----------------------------------------------------------------------------
# Host-side graph preprocessing (indices / layout only)
# ----------------------------------------------------------------------------
def _preprocess(edge_index, batch):
    row = np.asarray(edge_index[0], dtype=np.int64)
    col = np.asarray(edge_index[1], dtype=np.int64)
    # Self-loops are NOT in the slot structure: every dest has exactly one,
    # and its source row is the dest's own row in the core-LOCAL u table
    # (u1loc/u2loc) - added via a sequential DMA + one matmul per window.
    r_all = row
    c_all = col

    deg = np.bincount(c_all, minlength=N) + 1      # includes self-loop
    # global degree-descending order, dealt round-robin across cores: the
    # node of global degree-rank i lives on core i%8 at position i//8.
    # This balances per-window max degree across cores (slots are unified
    # over cores) and keeps windows degree-homogeneous.
    grank = np.empty(N, np.int64)
    gorder = np.argsort(-deg, kind="stable")
    grank[gorder] = np.arange(N)
    core_of = grank % NCORES
    pos = grank // NCORES
    gid = core_of * NP + pos                        # padded global row id

    # slot assignment: group edges by (dest core, dest sorted position)
    srcg = gid[r_all]
    dcore = core_of[c_all]
    dpos = pos[c_all]
    key = dcore * NP + dpos
    eorder = np.argsort(key, kind="stable")
    key_s = key[eorder]
    srcg_s = srcg[eorder]
    uniq, starts = np.unique(key_s, return_index=True)
    cnts = np.diff(np.append(starts, len(key_s)))
    k_within = np.arange(len(key_s)) - np.repeat(starts, cnts)

    u_core = uniq // NP
    u_w = (uniq % NP) // P
    Kmat = np.zeros((NCORES, W), np.int64)
    np.maximum.at(Kmat, (u_core, u_w), cnts)
    Kw = Kmat.max(axis=0)                           # [W] unified over cores
    off = np.concatenate([[0], np.cumsum(Kw)]).astype(np.int64)
    S = int(off[-1])

    e_core = key_s // NP
    e_pos = key_s % NP
    e_w = e_pos // P
    e_p = e_pos % P
    e_slice = off[e_w] + k_within

    offs = np.full((NCORES, P, S), DUMMY_ROW, np.int32)
    offs[e_core, e_p, e_slice] = srcg_s.astype(np.int32)

    graph_meta = dict(deg=deg, pos=pos, core_of=core_of, Kw=Kw, off=off, S=S)
    return offs, graph_meta


def _shards(inputs, meta):
    """Build per-core input arrays (layout transforms only)."""
    x = np.asarray(inputs["x"], np.float32)
    batch = np.asarray(inputs["batch"], np.int64)
    deg = meta["deg"]
    pos = meta["pos"]

    core_of = meta["core_of"]
    xs, degs, batchfs, cnts = [], [], [], []
    cnt = np.bincount(batch, minlength=G).astype(np.float32)
    for r in range(NCORES):
        nodes = np.nonzero(core_of == r)[0]
        pr = pos[nodes]
        xp = np.zeros((NP, H), np.float32)
        xp[pr] = x[nodes]
        xs.append(np.ascontiguousarray(xp.T))       # [128, NP] pre-transposed
        dg = np.full(NP, 1e20, np.float32)
        dg[pr] = deg[nodes].astype(np.float32)
        degs.append(np.ascontiguousarray(dg.reshape(W, P).T))   # [128, 49]
        bf = np.full(NP, 999.0, np.float32)
        bf[pr] = batch[nodes].astype(np.float32)
        batchfs.append(np.ascontiguousarray(bf.reshape(W, P).T))  # [128, 49]
        cnts.append(cnt[:, None])
    return xs, degs, batchfs, cnts


# ----------------------------------------------------------------------------
# Device program
# ----------------------------------------------------------------------------
def _build(meta, bias_zero, bench=None):
    """bench: None for the real kernel, or (phase, reps) to amplify one
    phase with a hardware loop / static unroll for wall-clock timing:
    ("agg", R) | ("gat", R) | ("red", R) | ("mm", R) | ("ag", R) | ("ar", R)."""
    Kw, off, S = meta["Kw"], meta["off"], meta["S"]
    b1_zero, b2_zero, bg_zero, bf_zero = bias_zero
    bench_phase, bench_reps = bench if bench else (None, 1)

    nc = bacc.Bacc("TRN2", target_bir_lowering=False, debug=False,
                   num_devices=NCORES)

    # ---- dram parameters -------------------------------------------------
    def din(name, shape, d=dt.float32):
        return nc.dram_tensor(name, shape, d, kind="ExternalInput")

    x_sh = din("x_sh", [H, NP])                     # x^T, feature-major
    w1 = din("w1", [H, H])
    w2 = din("w2", [H, H])
    wga = din("wga", [H, 256])
    wgb = din("wgb", [SV, 256])
    wfa = din("wfa", [H, A])
    wfb = din("wfb", [128, A])
    b1b = din("b1b", [P, H])
    b2b = din("b2b", [P, H])
    bgb = din("bgb", [G, 256])
    bfb = din("bfb", [G, A])
    degf = din("degf", [P, W])
    batchf = din("batchf", [P, W])
    cntv = din("cntv", [G, 1])
    svt = din("svt", [SV, G])
    iota64 = din("iota64", [P, G])
    ident = din("ident", [P, P])
    offs_in = din("offs", [P, S], dt.int32)
    out_ext = nc.dram_tensor("out", [G, A], dt.float32, kind="ExternalOutput")

    CC = list(range(NCORES))
    from contextlib import nullcontext

    with tile.TileContext(nc) as tc:
        def maybe_loop(phase):
            if bench_phase == phase and bench_reps > 1:
                return tc.For_i(0, bench_reps, 1)
            return nullcontext()

        with (
            tc.tile_pool(name="dram", bufs=1, space="DRAM") as dram,
            tc.tile_pool(name="const", bufs=1) as cp,
            tc.tile_pool(name="work", bufs=2) as wp,
            tc.tile_pool(name="gat", bufs=32) as gp,
            tc.tile_pool(name="res", bufs=1) as rp,
            tc.tile_pool(name="ps", bufs=2, space="PSUM") as pp,
            tc.tile_pool(name="ps1", bufs=1, space="PSUM") as pp1,
        ):
            # ---- DRAM internals ------------------------------------------
            u1loc = dram.tile([NP, H], dt.float32, name="u1loc")
            u1full = dram.tile([NT, H], dt.float32, addr_space="Shared",
                               name="u1full")
            u2loc = dram.tile([NP, H], dt.float32, name="u2loc")
            u2full = dram.tile([NT, H], dt.float32, addr_space="Shared",
                               name="u2full")
            pool_in = dram.tile([G, H], dt.float32, name="pool_in")
            pool_out = dram.tile([G, H], dt.float32, addr_space="Shared",
                                 name="pool_out")

            # ---- constants into SBUF -------------------------------------
            w1s = cp.tile([H, H], dt.float32, name="w1s")
            nc.sync.dma_start(w1s[:], w1[:])
            w2s = cp.tile([H, H], dt.float32, name="w2s")
            nc.sync.dma_start(w2s[:], w2[:])
            idents = cp.tile([P, P], dt.float32, name="idents")
            nc.sync.dma_start(idents[:], ident[:])
            iotas = cp.tile([P, G], dt.float32, name="iotas")
            nc.sync.dma_start(iotas[:], iota64[:])
            batchs = cp.tile([P, W], dt.float32, name="batchs")
            nc.sync.dma_start(batchs[:], batchf[:])
            offs = cp.tile([P, S], dt.int32, name="offs")
            nc.sync.dma_start(offs[:], offs_in[:])
            if not b1_zero:
                b1s = cp.tile([P, H], dt.float32, name="b1s")
                nc.sync.dma_start(b1s[:], b1b[:])
            if not b2_zero:
                b2s = cp.tile([P, H], dt.float32, name="b2s")
                nc.sync.dma_start(b2s[:], b2b[:])

            # dinv [128, W]: per-partition scale per window
            dinv = cp.tile([P, W], dt.float32, name="dinv")
            nc.sync.dma_start(dinv[:], degf[:])
            nc.vector.reciprocal(dinv[:], dinv[:])
            nc.scalar.activation(dinv[:], dinv[:],
                                 mybir.ActivationFunctionType.Sqrt)

            # ---- phase mm: u1 = (x @ W1) * dinv --------------------------
            # x arrives pre-transposed [H, NP]; each window's lhsT is a
            # direct strided load (no PE transpose / PSUM round-trip).
            MMB = 7                      # windows per batched x^T load/store
            with maybe_loop("mm"):
                for w in range(W):
                    j = w % MMB
                    if j == 0:
                        xt = wp.tile([P, MMB * P], dt.float32, name="xt",
                                     tag="xt")
                        nc.sync.dma_start(
                            xt[:], x_sh[:, w * P : (w + MMB) * P])
                        u1w = wp.tile([P, MMB, H], dt.float32, name="u1w",
                                      tag="u1w")
                    pu = pp.tile([P, H], dt.float32, name="pu", tag="mmA")
                    nc.tensor.matmul(pu[:], lhsT=xt[:, j * P : (j + 1) * P],
                                     rhs=w1s[:], start=True, stop=True)
                    nc.scalar.activation(u1w[:, j, :], pu[:],
                                         mybir.ActivationFunctionType.Copy,
                                         scale=dinv[:, w : w + 1])
                    if j == MMB - 1:
                        w0 = w - (MMB - 1)
                        nc.sync.dma_start(
                            u1loc[w0 * P : (w + 1) * P, :].rearrange(
                                "(j p) h -> p j h", j=MMB, p=P),
                            u1w[:])

            # ---- AllGather u1 --------------------------------------------
            if bench_phase == "ag":
                for bi in range(bench_reps - 1):
                    aux = dram.tile([NT, H], dt.float32, addr_space="Shared",
                                    name=f"agx{bi}")
                    nc.gpsimd.collective_compute(
                        "AllGather", mybir.AluOpType.bypass,
                        replica_groups=[CC],
                        ins=[u1loc.opt()], outs=[aux.opt()])
            nc.gpsimd.collective_compute(
                "AllGather", mybir.AluOpType.bypass, replica_groups=[CC],
                ins=[u1loc.opt()], outs=[u1full.opt()])

            def aggregate(ufull, uloc, layer, gat_only=False, red_only=False,
                          gm_only=False):
                """Edge aggregation + fused epilogue for one GCN layer."""
                bz = b1_zero if layer == 1 else b2_zero
                out_res = rp.tile([P, W, H], dt.float32, name="out_res",
                                  tag="out_res")
                # pass 1: per window, gather K_w slot tiles, sum on PE
                # into PSUM (+ the self-loop rows via a sequential DMA from
                # the core-local u table), apply relu(s*dinv [+ b]) into the
                # resident buf
                for w in range(W):
                    kw = 1 if red_only else int(Kw[w])
                    assert kw > 0
                    psw = pp.tile([P, H], dt.float32, name="psw", tag="psw",
                                  bufs=3)
                    if not gat_only:
                        st = wp.tile([P, H], dt.float32, name="st", tag="st",
                                     bufs=3)
                        nc.sync.dma_start(st[:], uloc[w * P : (w + 1) * P, :])
                        nc.tensor.matmul(psw[:], lhsT=idents[:], rhs=st[:],
                                         start=True, stop=False)
                    for k in range(kw):
                        sl = int(off[w]) + k
                        gt = gp.tile([P, H], dt.float32, name="gt", tag="gt")
                        nc.gpsimd.indirect_dma_start(
                            out=gt[:], out_offset=None, in_=ufull[:],
                            in_offset=bass.IndirectOffsetOnAxis(
                                ap=offs[:, sl : sl + 1], axis=0))
                        if not gat_only:
                            nc.tensor.matmul(psw[:], lhsT=idents[:],
                                             rhs=gt[:], start=False,
                                             stop=(k == kw - 1))
                    if gat_only:
                        continue
                    if bz:
                        nc.scalar.activation(
                            out_res[:, w, :], psw[:],
                            mybir.ActivationFunctionType.Relu,
                            scale=dinv[:, w : w + 1])
                    else:
                        ow = wp.tile([P, H], dt.float32, name="ow", tag="ow")
                        nc.vector.tensor_scalar(
                            ow[:], psw[:], dinv[:, w : w + 1], None,
                            op0=mybir.AluOpType.mult)
                        bs = b1s if layer == 1 else b2s
                        nc.vector.tensor_add(ow[:], ow[:], bs[:])
                        nc.scalar.activation(
                            out_res[:, w, :], ow[:],
                            mybir.ActivationFunctionType.Relu)

                    # epilogue for window w, overlapped with the next
                    # window's gathers
                    if layer == 1:
                        pto = pp.tile([P, P], dt.float32, name="pto",
                                      tag="mmA")
                        nc.tensor.transpose(pto[:], out_res[:, w, :],
                                            idents[:])
                        ots = wp.tile([P, P], dt.float32, name="ots",
                                      tag="xts")
                        nc.vector.tensor_copy(ots[:], pto[:])
                        pu2 = pp.tile([P, H], dt.float32, name="pu2",
                                      tag="mmA")
                        nc.tensor.matmul(pu2[:], lhsT=ots[:], rhs=w2s[:],
                                         start=True, stop=True)
                        u2w = wp.tile([P, H], dt.float32, name="u2w",
                                      tag="u2w")
                        nc.scalar.activation(
                            u2w[:], pu2[:],
                            mybir.ActivationFunctionType.Copy,
                            scale=dinv[:, w : w + 1])
                        nc.sync.dma_start(u2loc[w * P : (w + 1) * P, :],
                                          u2w[:])
                    else:
                        if w == 0:
                            ppool = pp1.tile([G, H], dt.float32,
                                             name="ppool", tag="head")
                        bw = wp.tile([P, G], dt.float32, name="bw", tag="bw")
                        nc.vector.tensor_scalar(
                            bw[:], iotas[:], batchs[:, w : w + 1], None,
                            op0=mybir.AluOpType.is_equal)
                        nc.tensor.matmul(ppool[:], lhsT=bw[:],
                                         rhs=out_res[:, w, :],
                                         start=(w == 0), stop=(w == W - 1))
                        if w == W - 1:
                            pools = cp.tile([G, H], dt.float32, name="pools")
                            nc.vector.tensor_copy(pools[:], ppool[:])
                            nc.sync.dma_start(pool_in[:], pools[:])

                if gat_only or gm_only:
                    if layer == 2:
                        pools = cp.tile([G, H], dt.float32, name="pools")
                        nc.vector.memset(pools[:], 0.0)
                        nc.sync.dma_start(pool_in[:], pools[:])
                    return

            gat_only = bench_phase == "gat"
            red_only = bench_phase == "red"
            gm_only = bench_phase == "gm"
            agg_loop = bench_phase if bench_phase in ("agg", "gat", "red",
                                                      "gm") else "agg"
            with maybe_loop(agg_loop):
                aggregate(u1full, u1loc, layer=1, gat_only=gat_only,
                          red_only=red_only, gm_only=gm_only)

            nc.gpsimd.collective_compute(
                "AllGather", mybir.AluOpType.bypass, replica_groups=[CC],
                ins=[u2loc.opt()], outs=[u2full.opt()])

            with maybe_loop(agg_loop):
                aggregate(u2full, u2loc, layer=2, gat_only=gat_only,
                          red_only=red_only, gm_only=gm_only)

            # ---- pooling AllReduce + MLP head ----------------------------
            if bench_phase == "ar":
                for bi in range(bench_reps - 1):
                    aux = dram.tile([G, H], dt.float32, addr_space="Shared",
                                    name=f"arx{bi}")
                    nc.gpsimd.collective_compute(
                        "AllReduce", mybir.AluOpType.add, replica_groups=[CC],
                        ins=[pool_in.opt()], outs=[aux.opt()])
            nc.gpsimd.collective_compute(
                "AllReduce", mybir.AluOpType.add, replica_groups=[CC],
                ins=[pool_in.opt()], outs=[pool_out.opt()])

            cnts = cp.tile([G, 1], dt.float32, name="cnts")
            nc.sync.dma_start(cnts[:], cntv[:])
            cinv = cp.tile([G, 1], dt.float32, name="cinv")
            nc.vector.reciprocal(cinv[:], cnts[:])

            pooled_raw = cp.tile([G, H], dt.float32, name="pooled_raw")
            nc.sync.dma_start(pooled_raw[:], pool_out[:])
            pooled = cp.tile([G, H], dt.float32, name="pooled")
            nc.vector.tensor_scalar(pooled[:], pooled_raw[:], cinv[:, 0:1],
                                    None, op0=mybir.AluOpType.mult)

            # pooled^T [128, 64]
            ptp = pp.tile([H, G], dt.float32, name="ptp", tag="mmA")
            nc.tensor.transpose(ptp[:], pooled[:], idents[:G, :G])
            pooledT = cp.tile([H, G], dt.float32, name="pooledT")
            nc.vector.tensor_copy(pooledT[:], ptp[:])

            svts = cp.tile([SV, G], dt.float32, name="svts")
            nc.sync.dma_start(svts[:], svt[:])
            wgas = cp.tile([H, 256], dt.float32, name="wgas")
            nc.sync.dma_start(wgas[:], wga[:])
            wgbs = cp.tile([SV, 256], dt.float32, name="wgbs")
            nc.sync.dma_start(wgbs[:], wgb[:])

            pz = pp1.tile([G, 256], dt.float32, name="pz", tag="head")
            nc.tensor.matmul(pz[:], lhsT=pooledT[:], rhs=wgas[:],
                             start=True, stop=False)
            nc.tensor.matmul(pz[:], lhsT=svts[:], rhs=wgbs[:],
                             start=False, stop=True)

            zs = cp.tile([G, 256], dt.float32, name="zs")
            if bg_zero:
                nc.scalar.activation(zs[:], pz[:],
                                     mybir.ActivationFunctionType.Relu)
            else:
                bgs = cp.tile([G, 256], dt.float32, name="bgs")
                nc.sync.dma_start(bgs[:], bgb[:])
                nc.vector.tensor_add(zs[:], pz[:], bgs[:])
                nc.scalar.activation(zs[:], zs[:],
                                     mybir.ActivationFunctionType.Relu)

            # z^T chunks [128, 64] x2
            wfas = cp.tile([H, A], dt.float32, name="wfas")
            nc.sync.dma_start(wfas[:], wfa[:])
            wfbs = cp.tile([128, A], dt.float32, name="wfbs")
            nc.sync.dma_start(wfbs[:], wfb[:])
            po = pp1.tile([G, A], dt.float32, name="po", tag="head")
            for ci in range(2):
                pzt = pp.tile([P, G], dt.float32, name="pzt", tag="mmA")
                nc.tensor.transpose(pzt[:], zs[:, ci * 128 : (ci + 1) * 128],
                                    idents[:G, :G])
                zts = wp.tile([P, G], dt.float32, name="zts", tag="zts")
                nc.vector.tensor_copy(zts[:], pzt[:])
                nc.tensor.matmul(po[:], lhsT=zts[:],
                                 rhs=(wfas[:] if ci == 0 else wfbs[:]),
                                 start=(ci == 0), stop=(ci == 1))

            outs = cp.tile([G, A], dt.float32, name="outs")
            if bf_zero:
                nc.scalar.activation(outs[:], po[:],
                                     mybir.ActivationFunctionType.Tanh)
            else:
                bfs = cp.tile([G, A], dt.float32, name="bfs")
                nc.sync.dma_start(bfs[:], bfb[:])
                nc.vector.tensor_add(outs[:], po[:], bfs[:])
                nc.scalar.activation(outs[:], outs[:],
                                     mybir.ActivationFunctionType.Tanh)
            nc.sync.dma_start(out_ext[:], outs[:])

    nc.compile()
    return nc


_CACHE = {}


def _get_program(inputs):
    edge_index = np.asarray(inputs["edge_index"])
    batch = np.asarray(inputs["batch"])
    bias_zero = tuple(
        bool(np.all(np.asarray(inputs[k]) == 0)) for k in ("b1", "b2", "bg", "bf")
    )
    import hashlib

    hkey = hashlib.sha256()
    hkey.update(edge_index.tobytes())
    hkey.update(batch.tobytes())
    hkey.update(repr(bias_zero).encode())
    ckey = hkey.hexdigest()
    if ckey not in _CACHE:
        offs, meta = _preprocess(edge_index, batch)
        nc = _build(meta, bias_zero)
        _CACHE[ckey] = (nc, offs, meta, bias_zero)
    return _CACHE[ckey]


def _in_maps(inputs, offs, meta):
    xs, degs, batchfs, cnts = _shards(inputs, meta)
    f32 = lambda a: np.ascontiguousarray(np.asarray(a, np.float32))
    W1 = f32(inputs["W1"])
    W2 = f32(inputs["W2"])
    Wg = f32(inputs["Wg"])
    Wf = f32(inputs["Wf"])
    b1 = f32(inputs["b1"])
    b2 = f32(inputs["b2"])
    bg = f32(inputs["bg"])
    bf = f32(inputs["bf"])
    sv = f32(inputs["state_vector"])
    common = dict(
        w1=W1, w2=W2,
        wga=f32(Wg[:H]), wgb=f32(Wg[H:]),
        wfa=f32(Wf[:H]), wfb=f32(Wf[H:]),
        b1b=np.tile(b1[None, :], (P, 1)),
        b2b=np.tile(b2[None, :], (P, 1)),
        bgb=np.tile(bg[None, :], (G, 1)),
        bfb=np.tile(bf[None, :], (G, 1)),
        svt=f32(sv.T),
        iota64=np.tile(np.arange(G, dtype=np.float32)[None, :], (P, 1)),
        ident=np.eye(P, dtype=np.float32),
    )
    maps = []
    for r in range(NCORES):
        m = dict(common)
        m["x_sh"] = xs[r]
        m["degf"] = degs[r]
        m["batchf"] = batchfs[r]
        m["cntv"] = cnts[r]
        m["offs"] = np.ascontiguousarray(offs[r])
        maps.append(m)
    return maps


def kernel(**inputs) -> np.ndarray:
    nc, offs, meta, _bz = _get_program(inputs)
    maps = _in_maps(inputs, offs, meta)
    res = run_bass_kernel_spmd(nc, maps, core_ids=list(range(NCORES)))
    return np.asarray(res.results[0]["out"], np.float32)



# revision 18
# speedup vs baseline: 1.0362x; 1.0362x over previous
"""Trainium2 Bass kernel for nn_Actor (2x GCNConv + mean-pool + MLP head).

Strategy (8 NeuronCores, SPMD):
  - Nodes are partitioned across the 8 cores (6250 each, padded to 6272).
    Within each core, local nodes are reordered by descending (in-)degree so
    that fixed-slot edge aggregation has minimal padding.
  - Node tensors are stored row-major [node, feature] (node on SBUF
    partitions for on-chip tiles).
  - Per layer: u = (h @ W) * dinv is computed locally, AllGathered so every
    core holds the full u table [50176, 128] f32 in DRAM.
  - Edge aggregation: destinations are processed in windows of 128 (one
    SBUF partition each). Edges into a window are organized into K_w
    "slots"; slot k is one indirect-DMA gather of 128 source rows (dest p
    <- u_full[src of its k-th edge], dummy = zero row). The K_w gathered
    tiles are summed on the TensorEngine by identity-matmul accumulation
    into PSUM - the only per-edge compute.
  - Epilogue fuses relu(s * dinv + b) (ACT, per-partition scale) -> next
    layer's matmul -> u2, and for the final layer the per-graph indicator
    matmul pooling (batch ids -> is_equal one-hots).
  - Pooled sums are AllReduced, the tiny MLP head runs replicated on every
    core; output [64, 8] is identical on all cores.

All index/layout preparation happens on the host (graph structure only);
all floating-point math runs on device.
"""
import sys

sys.path.insert(0, "/opt/trn_rl_repo")

import numpy as np

import concourse.bass as bass
import concourse.mybir as mybir
import concourse.tile as tile
from concourse import bacc
from concourse.bass_utils import run_bass_kernel_spmd

dt = mybir.dt

# Problem constants (hardcoded per spec)
N = 50000
E = 500000
H = 128
G = 64
SV = 64
A = 8
NCORES = 8
NLOC = N // NCORES          # 6250
P = 128
W = 49                      # windows of 128 dests per core
NP = W * P                  # 6272 padded local nodes
NT = NCORES * NP            # 50176 padded global rows in u_full
DUMMY_ROW = 6250            # core-0 pad row (always zero)


# ----------------------------------------------------------------------------
# Host-side graph preprocessing (indices / layout only)
# ----------------------------------------------------------------------------
def _preprocess(edge_index, batch):
    row = np.asarray(edge_index[0], dtype=np.int64)
    col = np.asarray(edge_index[1], dtype=np.int64)
    # Self-loops are NOT in the slot structure: every dest has exactly one,
    # and its source row is the dest's own row in the core-LOCAL u table
    # (u1loc/u2loc) - added via a sequential DMA + one matmul per window.
    r_all = row
    c_all = col

    deg = np.bincount(c_all, minlength=N) + 1      # includes self-loop
    # global degree-descending order, dealt round-robin across cores: the
    # node of global degree-rank i lives on core i%8 at position i//8.
    # This balances per-window max degree across cores (slots are unified
    # over cores) and keeps windows degree-homogeneous.
    grank = np.empty(N, np.int64)
    gorder = np.argsort(-deg, kind="stable")
    grank[gorder] = np.arange(N)
    core_of = grank % NCORES
    pos = grank // NCORES
    gid = core_of * NP + pos                        # padded global row id

    # slot assignment: group edges by (dest core, dest sorted position)
    srcg = gid[r_all]
    dcore = core_of[c_all]
    dpos = pos[c_all]
    key = dcore * NP + dpos
    eorder = np.argsort(key, kind="stable")
    key_s = key[eorder]
    srcg_s = srcg[eorder]
    uniq, starts = np.unique(key_s, return_index=True)
    cnts = np.diff(np.append(starts, len(key_s)))
    k_within = np.arange(len(key_s)) - np.repeat(starts, cnts)

    u_core = uniq // NP
    u_w = (uniq % NP) // P
    Kmat = np.zeros((NCORES, W), np.int64)
    np.maximum.at(Kmat, (u_core, u_w), cnts)
    Kw = Kmat.max(axis=0)                           # [W] unified over cores
    off = np.concatenate([[0], np.cumsum(Kw)]).astype(np.int64)
    S = int(off[-1])

    e_core = key_s // NP
    e_pos = key_s % NP
    e_w = e_pos // P
    e_p = e_pos % P
    e_slice = off[e_w] + k_within

    offs = np.full((NCORES, P, S), DUMMY_ROW, np.int32)
    offs[e_core, e_p, e_slice] = srcg_s.astype(np.int32)

    graph_meta = dict(deg=deg, pos=pos, core_of=core_of, Kw=Kw, off=off, S=S)
    return offs, graph_meta


def _shards(inputs, meta):
    """Build per-core input arrays (layout transforms only)."""
    x = np.asarray(inputs["x"], np.float32)
    batch = np.asarray(inputs["batch"], np.int64)
    deg = meta["deg"]
    pos = meta["pos"]

    core_of = meta["core_of"]
    xs, degs, batchfs, cnts = [], [], [], []
    cnt = np.bincount(batch, minlength=G).astype(np.float32)
    for r in range(NCORES):
        nodes = np.nonzero(core_of == r)[0]
        pr = pos[nodes]
        xp = np.zeros((NP, H), np.float32)
        xp[pr] = x[nodes]
        xs.append(np.ascontiguousarray(xp.T))       # [128, NP] pre-transposed
        dg = np.full(NP, 1e20, np.float32)
        dg[pr] = deg[nodes].astype(np.float32)
        degs.append(np.ascontiguousarray(dg.reshape(W, P).T))   # [128, 49]
        bf = np.full(NP, 999.0, np.float32)
        bf[pr] = batch[nodes].astype(np.float32)
        batchfs.append(np.ascontiguousarray(bf.reshape(W, P).T))  # [128, 49]
        cnts.append(cnt[:, None])
    return xs, degs, batchfs, cnts


# ----------------------------------------------------------------------------
# Device program
# ----------------------------------------------------------------------------
def _build(meta, bias_zero, bench=None):
    """bench: None for the real kernel, or (phase, reps) to amplify one
    phase with a hardware loop / static unroll for wall-clock timing:
    ("agg", R) | ("gat", R) | ("red", R) | ("mm", R) | ("ag", R) | ("ar", R)."""
    Kw, off, S = meta["Kw"], meta["off"], meta["S"]
    b1_zero, b2_zero, bg_zero, bf_zero = bias_zero
    bench_phase, bench_reps = bench if bench else (None, 1)

    nc = bacc.Bacc("TRN2", target_bir_lowering=False, debug=False,
                   num_devices=NCORES)

    # ---- dram parameters -------------------------------------------------
    def din(name, shape, d=dt.float32):
        return nc.dram_tensor(name, shape, d, kind="ExternalInput")

    x_sh = din("x_sh", [H, NP])                     # x^T, feature-major
    w1 = din("w1", [H, H])
    w2 = din("w2", [H, H])
    wga = din("wga", [H, 256])
    wgb = din("wgb", [SV, 256])
    wfa = din("wfa", [H, A])
    wfb = din("wfb", [128, A])
    b1b = din("b1b", [P, H])
    b2b = din("b2b", [P, H])
    bgb = din("bgb", [G, 256])
    bfb = din("bfb", [G, A])
    degf = din("degf", [P, W])
    batchf = din("batchf", [P, W])
    cntv = din("cntv", [G, 1])
    svt = din("svt", [SV, G])
    iota64 = din("iota64", [P, G])
    ident = din("ident", [P, P])
    offs_in = din("offs", [P, S], dt.int32)
    out_ext = nc.dram_tensor("out", [G, A], dt.float32, kind="ExternalOutput")

    CC = list(range(NCORES))
    from contextlib import nullcontext

    with tile.TileContext(nc) as tc:
        def maybe_loop(phase):
            if bench_phase == phase and bench_reps > 1:
                return tc.For_i(0, bench_reps, 1)
            return nullcontext()

        with (
            tc.tile_pool(name="dram", bufs=1, space="DRAM") as dram,
            tc.tile_pool(name="const", bufs=1) as cp,
            tc.tile_pool(name="work", bufs=2) as wp,
            tc.tile_pool(name="gat", bufs=32) as gp,
            tc.tile_pool(name="res", bufs=1) as rp,
            tc.tile_pool(name="ps", bufs=2, space="PSUM") as pp,
            tc.tile_pool(name="ps1", bufs=1, space="PSUM") as pp1,
        ):
            # ---- DRAM internals ------------------------------------------
            u1loc = dram.tile([NP, H], dt.float32, name="u1loc")
            u1full = dram.tile([NT, H], dt.float32, addr_space="Shared",
                               name="u1full")
            u2loc = dram.tile([NP, H], dt.float32, name="u2loc")
            u2full = dram.tile([NT, H], dt.float32, addr_space="Shared",
                               name="u2full")
            pool_in = dram.tile([G, H], dt.float32, name="pool_in")
            pool_out = dram.tile([G, H], dt.float32, addr_space="Shared",
                                 name="pool_out")

            # ---- constants into SBUF -------------------------------------
            w1s = cp.tile([H, H], dt.float32, name="w1s")
            nc.sync.dma_start(w1s[:], w1[:])
            w2s = cp.tile([H, H], dt.float32, name="w2s")
            nc.sync.dma_start(w2s[:], w2[:])
            idents = cp.tile([P, P], dt.float32, name="idents")
            nc.sync.dma_start(idents[:], ident[:])
            iotas = cp.tile([P, G], dt.float32, name="iotas")
            nc.sync.dma_start(iotas[:], iota64[:])
            batchs = cp.tile([P, W], dt.float32, name="batchs")
            nc.sync.dma_start(batchs[:], batchf[:])
            offs = cp.tile([P, S], dt.int32, name="offs")
            nc.sync.dma_start(offs[:], offs_in[:])
            if not b1_zero:
                b1s = cp.tile([P, H], dt.float32, name="b1s")
                nc.sync.dma_start(b1s[:], b1b[:])
            if not b2_zero:
                b2s = cp.tile([P, H], dt.float32, name="b2s")
                nc.sync.dma_start(b2s[:], b2b[:])

            # dinv [128, W]: per-partition scale per window
            dinv = cp.tile([P, W], dt.float32, name="dinv")
            nc.sync.dma_start(dinv[:], degf[:])
            nc.vector.reciprocal(dinv[:], dinv[:])
            nc.scalar.activation(dinv[:], dinv[:],
                                 mybir.ActivationFunctionType.Sqrt)

            # ---- phase mm: u1 = (x @ W1) * dinv --------------------------
            # x arrives pre-transposed [H, NP]; each window's lhsT is a
            # direct strided load (no PE transpose / PSUM round-trip).
            MMB = 7                      # windows per batched x^T load/store
            with maybe_loop("mm"):
                for w in range(W):
                    j = w % MMB
                    if j == 0:
                        xt = wp.tile([P, MMB * P], dt.float32, name="xt",
                                     tag="xt")
                        nc.sync.dma_start(
                            xt[:], x_sh[:, w * P : (w + MMB) * P])
                        u1w = wp.tile([P, MMB, H], dt.float32, name="u1w",
                                      tag="u1w")
                    pu = pp.tile([P, H], dt.float32, name="pu", tag="mmA")
                    nc.tensor.matmul(pu[:], lhsT=xt[:, j * P : (j + 1) * P],
                                     rhs=w1s[:], start=True, stop=True)
                    nc.scalar.activation(u1w[:, j, :], pu[:],
                                         mybir.ActivationFunctionType.Copy,
                                         scale=dinv[:, w : w + 1])
                    if j == MMB - 1:
                        w0 = w - (MMB - 1)
                        nc.sync.dma_start(
                            u1loc[w0 * P : (w + 1) * P, :].rearrange(
                                "(j p) h -> p j h", j=MMB, p=P),
                            u1w[:])

            # ---- AllGather u1 --------------------------------------------
            if bench_phase == "ag":
                for bi in range(bench_reps - 1):
                    aux = dram.tile([NT, H], dt.float32, addr_space="Shared",
                                    name=f"agx{bi}")
                    nc.gpsimd.collective_compute(
                        "AllGather", mybir.AluOpType.bypass,
                        replica_groups=[CC],
                        ins=[u1loc.opt()], outs=[aux.opt()])
            nc.gpsimd.collective_compute(
                "AllGather", mybir.AluOpType.bypass, replica_groups=[CC],
                ins=[u1loc.opt()], outs=[u1full.opt()])

            def aggregate(ufull, uloc, layer, gat_only=False, red_only=False,
                          gm_only=False):
                """Edge aggregation + fused epilogue for one GCN layer."""
                bz = b1_zero if layer == 1 else b2_zero
                out_res = rp.tile([P, W, H], dt.float32, name="out_res",
                                  tag="out_res")
                # pass 1: per window, gather K_w slot tiles, sum on PE
                # into PSUM (+ the self-loop rows via a sequential DMA from
                # the core-local u table, prefetched 2 windows ahead), apply
                # relu(s*dinv [+ b]) into the resident buf
                sts = {}

                def load_st(w):
                    if gat_only or w >= W or w in sts:
                        return
                    st = wp.tile([P, H], dt.float32, name="st", tag="st",
                                 bufs=3)
                    nc.sync.dma_start(st[:], uloc[w * P : (w + 1) * P, :])
                    sts[w] = st

                load_st(0)
                load_st(1)
                for w in range(W):
                    load_st(w + 2)
                    kw = 1 if red_only else int(Kw[w])
                    assert kw > 0
                    psw = pp.tile([P, H], dt.float32, name="psw", tag="psw",
                                  bufs=3)
                    if not gat_only:
                        nc.tensor.matmul(psw[:], lhsT=idents[:],
                                         rhs=sts.pop(w)[:],
                                         start=True, stop=False)
                    for k in range(kw):
                        sl = int(off[w]) + k
                        gt = gp.tile([P, H], dt.float32, name="gt", tag="gt")
                        nc.gpsimd.indirect_dma_start(
                            out=gt[:], out_offset=None, in_=ufull[:],
                            in_offset=bass.IndirectOffsetOnAxis(
                                ap=offs[:, sl : sl + 1], axis=0))
                        if not gat_only:
                            nc.tensor.matmul(psw[:], lhsT=idents[:],
                                             rhs=gt[:], start=False,
                                             stop=(k == kw - 1))
                    if gat_only:
                        continue
                    if bz:
                        nc.scalar.activation(
                            out_res[:, w, :], psw[:],
                            mybir.ActivationFunctionType.Relu,
                            scale=dinv[:, w : w + 1])
                    else:
                        ow = wp.tile([P, H], dt.float32, name="ow", tag="ow")
                        nc.vector.tensor_scalar(
                            ow[:], psw[:], dinv[:, w : w + 1], None,
                            op0=mybir.AluOpType.mult)
                        bs = b1s if layer == 1 else b2s
                        nc.vector.tensor_add(ow[:], ow[:], bs[:])
                        nc.scalar.activation(
                            out_res[:, w, :], ow[:],
                            mybir.ActivationFunctionType.Relu)

                    # epilogue for window w, overlapped with the next
                    # window's gathers
                    if layer == 1:
                        pto = pp.tile([P, P], dt.float32, name="pto",
                                      tag="mmA")
                        nc.tensor.transpose(pto[:], out_res[:, w, :],
                                            idents[:])
                        ots = wp.tile([P, P], dt.float32, name="ots",
                                      tag="xts")
                        nc.vector.tensor_copy(ots[:], pto[:])
                        pu2 = pp.tile([P, H], dt.float32, name="pu2",
                                      tag="mmA")
                        nc.tensor.matmul(pu2[:], lhsT=ots[:], rhs=w2s[:],
                                         start=True, stop=True)
                        u2w = wp.tile([P, H], dt.float32, name="u2w",
                                      tag="u2w")
                        nc.scalar.activation(
                            u2w[:], pu2[:],
                            mybir.ActivationFunctionType.Copy,
                            scale=dinv[:, w : w + 1])
                        nc.sync.dma_start(u2loc[w * P : (w + 1) * P, :],
                                          u2w[:])
                    else:
                        if w == 0:
                            ppool = pp1.tile([G, H], dt.float32,
                                             name="ppool", tag="head")
                        bw = wp.tile([P, G], dt.float32, name="bw", tag="bw")
                        nc.vector.tensor_scalar(
                            bw[:], iotas[:], batchs[:, w : w + 1], None,
                            op0=mybir.AluOpType.is_equal)
                        nc.tensor.matmul(ppool[:], lhsT=bw[:],
                                         rhs=out_res[:, w, :],
                                         start=(w == 0), stop=(w == W - 1))
                        if w == W - 1:
                            pools = cp.tile([G, H], dt.float32, name="pools")
                            nc.vector.tensor_copy(pools[:], ppool[:])
                            nc.sync.dma_start(pool_in[:], pools[:])

                if gat_only or gm_only:
                    if layer == 2:
                        pools = cp.tile([G, H], dt.float32, name="pools")
                        nc.vector.memset(pools[:], 0.0)
                        nc.sync.dma_start(pool_in[:], pools[:])
                    return

            gat_only = bench_phase == "gat"
            red_only = bench_phase == "red"
            gm_only = bench_phase == "gm"
            agg_loop = bench_phase if bench_phase in ("agg", "gat", "red",
                                                      "gm") else "agg"
            with maybe_loop(agg_loop):
                aggregate(u1full, u1loc, layer=1, gat_only=gat_only,
                          red_only=red_only, gm_only=gm_only)

            nc.gpsimd.collective_compute(
                "AllGather", mybir.AluOpType.bypass, replica_groups=[CC],
                ins=[u2loc.opt()], outs=[u2full.opt()])

            with maybe_loop(agg_loop):
                aggregate(u2full, u2loc, layer=2, gat_only=gat_only,
                          red_only=red_only, gm_only=gm_only)

            # ---- pooling AllReduce + MLP head ----------------------------
            if bench_phase == "ar":
                for bi in range(bench_reps - 1):
                    aux = dram.tile([G, H], dt.float32, addr_space="Shared",
                                    name=f"arx{bi}")
                    nc.gpsimd.collective_compute(
                        "AllReduce", mybir.AluOpType.add, replica_groups=[CC],
                        ins=[pool_in.opt()], outs=[aux.opt()])
            nc.gpsimd.collective_compute(
                "AllReduce", mybir.AluOpType.add, replica_groups=[CC],
                ins=[pool_in.opt()], outs=[pool_out.opt()])

            cnts = cp.tile([G, 1], dt.float32, name="cnts")
            nc.sync.dma_start(cnts[:], cntv[:])
            cinv = cp.tile([G, 1], dt.float32, name="cinv")
            nc.vector.reciprocal(cinv[:], cnts[:])

            pooled_raw = cp.tile([G, H], dt.float32, name="pooled_raw")
            nc.sync.dma_start(pooled_raw[:], pool_out[:])
            pooled = cp.tile([G, H], dt.float32, name="pooled")
            nc.vector.tensor_scalar(pooled[:], pooled_raw[:], cinv[:, 0:1],
                                    None, op0=mybir.AluOpType.mult)

            # pooled^T [128, 64]
            ptp = pp.tile([H, G], dt.float32, name="ptp", tag="mmA")
            nc.tensor.transpose(ptp[:], pooled[:], idents[:G, :G])
            pooledT = cp.tile([H, G], dt.float32, name="pooledT")
            nc.vector.tensor_copy(pooledT[:], ptp[:])

            svts = cp.tile([SV, G], dt.float32, name="svts")
            nc.sync.dma_start(svts[:], svt[:])
            wgas = cp.tile([H, 256], dt.float32, name="wgas")
            nc.sync.dma_start(wgas[:], wga[:])
            wgbs = cp.tile([SV, 256], dt.float32, name="wgbs")
            nc.sync.dma_start(wgbs[:], wgb[:])

            pz = pp1.tile([G, 256], dt.float32, name="pz", tag="head")
            nc.tensor.matmul(pz[:], lhsT=pooledT[:], rhs=wgas[:],
                             start=True, stop=False)
            nc.tensor.matmul(pz[:], lhsT=svts[:], rhs=wgbs[:],
                             start=False, stop=True)

            zs = cp.tile([G, 256], dt.float32, name="zs")
            if bg_zero:
                nc.scalar.activation(zs[:], pz[:],
                                     mybir.ActivationFunctionType.Relu)
            else:
                bgs = cp.tile([G, 256], dt.float32, name="bgs")
                nc.sync.dma_start(bgs[:], bgb[:])
                nc.vector.tensor_add(zs[:], pz[:], bgs[:])
                nc.scalar.activation(zs[:], zs[:],
                                     mybir.ActivationFunctionType.Relu)

            # z^T chunks [128, 64] x2
            wfas = cp.tile([H, A], dt.float32, name="wfas")
            nc.sync.dma_start(wfas[:], wfa[:])
            wfbs = cp.tile([128, A], dt.float32, name="wfbs")
            nc.sync.dma_start(wfbs[:], wfb[:])
            po = pp1.tile([G, A], dt.float32, name="po", tag="head")
            for ci in range(2):
                pzt = pp.tile([P, G], dt.float32, name="pzt", tag="mmA")
                nc.tensor.transpose(pzt[:], zs[:, ci * 128 : (ci + 1) * 128],
                                    idents[:G, :G])
                zts = wp.tile([P, G], dt.float32, name="zts", tag="zts")
                nc.vector.tensor_copy(zts[:], pzt[:])
                nc.tensor.matmul(po[:], lhsT=zts[:],
                                 rhs=(wfas[:] if ci == 0 else wfbs[:]),
                                 start=(ci == 0), stop=(ci == 1))

            outs = cp.tile([G, A], dt.float32, name="outs")
            if bf_zero:
                nc.scalar.activation(outs[:], po[:],
                                     mybir.ActivationFunctionType.Tanh)
            else:
                bfs = cp.tile([G, A], dt.float32, name="bfs")
                nc.sync.dma_start(bfs[:], bfb[:])
                nc.vector.tensor_add(outs[:], po[:], bfs[:])
                nc.scalar.activation(outs[:], outs[:],
                                     mybir.ActivationFunctionType.Tanh)
            nc.sync.dma_start(out_ext[:], outs[:])

    nc.compile()
    return nc


_CACHE = {}


def _get_program(inputs):
    edge_index = np.asarray(inputs["edge_index"])
    batch = np.asarray(inputs["batch"])
    bias_zero = tuple(
        bool(np.all(np.asarray(inputs[k]) == 0)) for k in ("b1", "b2", "bg", "bf")
    )
    import hashlib

    hkey = hashlib.sha256()
    hkey.update(edge_index.tobytes())
    hkey.update(batch.tobytes())
    hkey.update(repr(bias_zero).encode())
    ckey = hkey.hexdigest()
    if ckey not in _CACHE:
        offs, meta = _preprocess(edge_index, batch)
        nc = _build(meta, bias_zero)
        _CACHE[ckey] = (nc, offs, meta, bias_zero)
    return _CACHE[ckey]


def _in_maps(inputs, offs, meta):
    xs, degs, batchfs, cnts = _shards(inputs, meta)
    f32 = lambda a: np.ascontiguousarray(np.asarray(a, np.float32))
    W1 = f32(inputs["W1"])
    W2 = f32(inputs["W2"])
    Wg = f32(inputs["Wg"])
    Wf = f32(inputs["Wf"])
    b1 = f32(inputs["b1"])
    b2 = f32(inputs["b2"])
    bg = f32(inputs["bg"])
    bf = f32(inputs["bf"])
    sv = f32(inputs["state_vector"])
    common = dict(
        w1=W1, w2=W2,
        wga=f32(Wg[:H]), wgb=f32(Wg[H:]),
        wfa=f32(Wf[:H]), wfb=f32(Wf[H:]),
        b1b=np.tile(b1[None, :], (P, 1)),
        b2b=np.tile(b2[None, :], (P, 1)),
        bgb=np.tile(bg[None, :], (G, 1)),
        bfb=np.tile(bf[None, :], (G, 1)),
        svt=f32(sv.T),
        iota64=np.tile(np.arange(G, dtype=np.float32)[None, :], (P, 1)),
        ident=np.eye(P, dtype=np.float32),
    )
    maps = []
    for r in range(NCORES):
        m = dict(common)
        m["x_sh"] = xs[r]
        m["degf"] = degs[r]
        m["batchf"] = batchfs[r]
        m["cntv"] = cnts[r]
        m["offs"] = np.ascontiguousarray(offs[r])
        maps.append(m)
    return maps


def kernel(**inputs) -> np.ndarray:
    nc, offs, meta, _bz = _get_program(inputs)
    maps = _in_maps(inputs, offs, meta)
    res = run_bass_kernel_spmd(nc, maps, core_ids=list(range(NCORES)))
    return np.asarray(res.results[0]["out"], np.float32)

